# revision 2
# baseline (speedup 1.0000x reference)
"""Additive (Bahdanau-style) attention scores kernel for Trainium2.

Computes softmax(We @ tanh(query@Wq.T + keys@Wk.T), axis=-1) for
B=32, S=2048, D=1024, data-parallel over batch across 8 NeuronCores.

v9: fp8 DoubleRow pk datapath (see v7) + fast-start and fast-tail.
  - weight loads split across the scalar and sync HWDGE queues; wqT
    arrives in halves so the pq matmuls (which double as the HAM
    warmup) start ~10us earlier; tiny qT warmup matmuls before that
  - keys staging chain is no longer seeded on the wk8 load: the first
    SWDGE cast issues right after the framework preamble
  - We-contraction col-tiled 4-way (tile_position via psum row 32j):
    2 concurrent rounds instead of 8 serial rank-1 matmuls per block
  - softmax without max-subtraction (|scores| <= 32 so exp is f32-
    safe): per-block exp on ACT with accum_out partial sums; the tail
    is only reduce(4)+reciprocal+scale+DMA for the last batch
"""

import numpy as np
import ml_dtypes
from contextlib import ExitStack

import concourse.bass as bass
import concourse.mybir as mybir
import concourse.tile as tile
from concourse import bacc
from concourse.bass_utils import run_bass_kernel_spmd
from concourse.masks import make_identity
from concourse.tile_rust import add_dep_helper

f32 = mybir.dt.float32
fp16 = mybir.dt.float16
fp8 = mybir.dt.float8e4
e4m3 = ml_dtypes.float8_e4m3

B, S, D, E = 32, 2048, 1024, 1024
NCORES = 8
BL = B // NCORES      # 4 batches per core
S_BLK = 512
N_SBLK = S // S_BLK   # 4
DT = D // 128         # 8 d-tiles
D2T = D // 256        # 4 d-pair chunks (256 contraction each)
ET = E // 128         # 8 e-tiles
WSC = 2.0 ** 12       # host-side Wk scale (fp8 range), undone in tanh

_CACHE: dict = {}


def _build_nc():
    nc = bacc.Bacc("TRN2", target_bir_lowering=False, debug=False, num_devices=NCORES)

    keys_d = nc.dram_tensor("keys", [BL, S, D], f32, kind="ExternalInput")
    keys8_d = nc.dram_tensor("keys8", [BL, S, D], fp8, kind="Internal")
    qT_d = nc.dram_tensor("queryT16", [128, DT, BL], fp16, kind="ExternalInput")
    wqT0_d = nc.dram_tensor("wqT16h0", [128, DT, 512], fp16, kind="ExternalInput")
    wqT1_d = nc.dram_tensor("wqT16h1", [128, DT, 512], fp16, kind="ExternalInput")
    wk8_d = nc.dram_tensor("wk8", [128, D2T, 2, E], fp8, kind="ExternalInput")
    weT_d = nc.dram_tensor("weT16", [128, ET], fp16, kind="ExternalInput")
    out_d = nc.dram_tensor("out", [BL, S], f32, kind="ExternalOutput")

    with tile.TileContext(nc) as tc, ExitStack() as ctx:
        wpool = ctx.enter_context(tc.tile_pool(name="weights", bufs=1))
        kT_pool = ctx.enter_context(tc.tile_pool(name="kT", bufs=3))
        en_pool = ctx.enter_context(tc.tile_pool(name="en", bufs=12))
        small = ctx.enter_context(tc.tile_pool(name="small", bufs=1))
        sm_pool = ctx.enter_context(tc.tile_pool(name="smx", bufs=2))

        ps_pk = ctx.enter_context(tc.tile_pool(name="ps_pk", bufs=3, space="PSUM"))
        ps_sc = ctx.enter_context(tc.tile_pool(name="ps_sc", bufs=2, space="PSUM"))
        ps_pq = ctx.enter_context(tc.tile_pool(name="ps_pq", bufs=1, space="PSUM"))
        ps_warm = ctx.enter_context(tc.tile_pool(name="ps_warm", bufs=1, space="PSUM"))

        # ---- weight loads spread over both HWDGE queues ----
        qT_sb = wpool.tile([128, DT, BL], fp16)
        nc.scalar.dma_start(qT_sb, qT_d[:])
        wqT_sb = wpool.tile([128, DT, E], fp16)
        wq_loads = [
            nc.scalar.dma_start(wqT_sb[:, :, 0:512], wqT0_d[:]),
            nc.scalar.dma_start(wqT_sb[:, :, 512:1024], wqT1_d[:]),
        ]
        wk8_sb = wpool.tile([128, D2T, 2, E], fp8)
        nc.sync.dma_start(wk8_sb, wk8_d[:])
        weT_sb = wpool.tile([128, ET], fp16)
        nc.sync.dma_start(weT_sb, weT_d[:])

        # ---- tiny PE warmup on qT while wqT/wk8 are in flight ----
        warm_ps = ps_warm.tile([BL, 32], f32)
        for w in range(40):
            nc.tensor.matmul(warm_ps,
                             lhsT=qT_sb[:, 0],
                             rhs=qT_sb[:].rearrange("p a b -> p (a b)"),
                             start=True, stop=True, skip_group_check=True)

        # ---- pq: layout-A matmul [b, e], then tiny PE transposes to [e, b] ----
        ident4 = wpool.tile([BL, BL], fp16)
        make_identity(nc, ident4)
        pq_row = wpool.tile([BL, E], fp16)
        for half in range(2):
            pq_ps = ps_pq.tile([BL, 512], f32, tag="pq_mm")
            for dt in range(DT):
                nc.tensor.matmul(pq_ps,
                                 lhsT=qT_sb[:, dt],
                                 rhs=wqT_sb[:, dt, half * 512 : (half + 1) * 512],
                                 start=(dt == 0), stop=(dt == DT - 1))
            nc.vector.tensor_copy(pq_row[:, half * 512 : (half + 1) * 512], pq_ps)
        pq_sb = wpool.tile([128, ET, BL], fp16)
        for et in range(ET):
            pq_tp = ps_pq.tile([128, BL], fp16, tag="pq_tr")
            nc.tensor.transpose(pq_tp, pq_row[:, et * 128 : (et + 1) * 128], ident4)
            nc.vector.tensor_copy(pq_sb[:, et], pq_tp)

        # exp(scores) rows (one partition; per-batch slices) + partial sums
        ex_tmp = small.tile([1, BL * S], f32)
        s_parts = small.tile([1, BL * N_SBLK], f32)

        def emit_tail(b):
            """Finish softmax of batch b: sum partials, scale, store."""
            ssum = sm_pool.tile([1, 1], f32, tag="ssum")
            nc.vector.reduce_sum(ssum, s_parts[0:1, b * N_SBLK : (b + 1) * N_SBLK],
                                 axis=mybir.AxisListType.X)
            rinv = sm_pool.tile([1, 1], f32, tag="rinv")
            nc.vector.reciprocal(rinv, ssum)
            outr = sm_pool.tile([1, S], f32, tag="outr")
            nc.vector.tensor_scalar_mul(outr, ex_tmp[0:1, b * S : (b + 1) * S], rinv)
            nc.scalar.dma_start(out_d[b : b + 1, :], outr)

        # ---- main loop over (batch, s-block) ----
        pending = None  # deferred We-contraction of the previous block

        blocks = [(b, sblk) for b in range(BL) for sblk in range(N_SBLK)]
        # group blocks for DRAM-staged fp8 keys: small groups first for a
        # fast ramp, then 2-block groups (few, big serialized DMA ops)
        groups = [[0], [1]] + [[i, i + 1] for i in range(2, len(blocks), 2)]
        blk_group = {}
        for gi, g in enumerate(groups):
            for bi_ in g:
                blk_group[bi_] = gi
        kT_tiles = {}
        last_xbar = None

        def emit_group(gi):
            nonlocal last_xbar
            g = groups[gi]
            bi0, (b0_, sblk0_) = g[0], blocks[g[0]]
            flat0 = b0_ * S + sblk0_ * S_BLK
            n_s = S_BLK * len(g)
            flat = keys_d[:].rearrange("b s d -> (b s) d")
            flat8 = keys8_d[:].rearrange("b s d -> (b s) d")
            c = nc.gpsimd.dma_start(flat8[flat0 : flat0 + n_s, :],
                                    flat[flat0 : flat0 + n_s, :])
            if last_xbar is not None:
                add_dep_helper(c.ins, last_xbar.ins, sync=True,
                               reason="batch DMA modes")
            # XBAR moves 16-bit containers: view fp8 pairs as fp16
            flat8_16 = flat8[flat0 : flat0 + n_s, :].bitcast(fp16)
            kT = kT_pool.tile([128, D2T, n_s], fp16,
                              tag=f"kT_{len(g)}")
            x = nc.sync.dma_start_transpose(kT, flat8_16)
            for off_, bi_ in enumerate(g):
                kT_tiles[bi_] = (kT, off_ * S_BLK)
            last_xbar = x

        emit_group(0)
        emit_group(1)

        for bi, (b, sblk) in enumerate(blocks):
            if bi + 2 < len(blocks):
                gi = blk_group[bi + 2]
                if min(groups[gi]) == bi + 2:
                    emit_group(gi)
            kT, s_off = kT_tiles.pop(bi)
            # fp8 element view: [128, D2T, 2*n_s]; per chunk c the pair
            # dim is the byte within a container (stride 1), s stride 2
            kT8 = kT[:].bitcast(fp8)

            en_tiles = []
            for et in range(ET):
                pk_ps = ps_pk.tile([128, S_BLK], f32)
                for c in range(D2T):
                    rhs = kT8[:, c, :].rearrange("p (s i) -> p i s", i=2)
                    rhs = rhs[:, :, s_off : s_off + S_BLK]
                    nc.tensor.matmul(
                        pk_ps,
                        lhsT=wk8_sb[:, c, :, et * 128 : (et + 1) * 128],
                        rhs=rhs,
                        start=(c == 0), stop=(c == D2T - 1),
                        perf_mode=mybir.MatmulPerfMode.DoubleRow)
                en = en_pool.tile([128, S_BLK], fp16)
                nc.scalar.activation(en, pk_ps,
                                     mybir.ActivationFunctionType.Tanh,
                                     bias=pq_sb[:, et, b : b + 1],
                                     scale=1.0 / WSC)
                en_tiles.append(en)
                if et == 1 and pending is not None:
                    pending()
                    pending = None

            def make_pending(b_, sblk_, tiles):
                def emit():
                    # 4-way col-tiled We-contraction: psum rows 0/32/64/96
                    sc_ps = ps_sc.tile([128, S_BLK], f32)
                    for et_ in range(ET):
                        j = et_ % 4
                        nc.tensor.matmul(sc_ps[32 * j : 32 * j + 1, :],
                                         lhsT=weT_sb[:, et_ : et_ + 1],
                                         rhs=tiles[et_],
                                         start=(et_ < 4), stop=(et_ >= 4),
                                         tile_position=(0, 32 * j),
                                         skip_group_check=True)
                    # sum the 4 col-tile rows; only one PSUM input per op
                    add = mybir.AluOpType.add
                    mult = mybir.AluOpType.mult
                    t0 = sm_pool.tile([1, S_BLK], f32, tag="t0")
                    nc.vector.tensor_copy(t0, sc_ps[0:1, :])
                    t1 = sm_pool.tile([1, S_BLK], f32, tag="t1")
                    nc.vector.scalar_tensor_tensor(
                        t1, t0, 1.0, sc_ps[32:33, :], op0=mult, op1=add)
                    t2 = sm_pool.tile([1, S_BLK], f32, tag="t2")
                    nc.vector.scalar_tensor_tensor(
                        t2, t1, 1.0, sc_ps[64:65, :], op0=mult, op1=add)
                    srow = sm_pool.tile([1, S_BLK], f32, tag="srow")
                    nc.vector.scalar_tensor_tensor(
                        srow, t2, 1.0, sc_ps[96:97, :], op0=mult, op1=add)
                    off = b_ * S + sblk_ * S_BLK
                    idx = b_ * N_SBLK + sblk_
                    nc.scalar.activation(
                        ex_tmp[0:1, off : off + S_BLK], srow,
                        mybir.ActivationFunctionType.Exp,
                        scale=1.0,
                        accum_out=s_parts[0:1, idx : idx + 1])
                    if sblk_ == N_SBLK - 1:
                        emit_tail(b_)
                return emit

            pending = make_pending(b, sblk, en_tiles)

        pending()

    nc.compile()
    return nc


def _get_nc():
    if "nc" not in _CACHE:
        _CACHE["nc"] = _build_nc()
    return _CACHE["nc"]


def kernel(query, keys, Wq, Wk, We, _return_raw=False, _trace=False):
    query = np.asarray(query, dtype=np.float32)
    keys = np.asarray(keys, dtype=np.float32)
    Wq = np.asarray(Wq, dtype=np.float32)
    Wk = np.asarray(Wk, dtype=np.float32)
    We = np.asarray(We, dtype=np.float32)

    # pack [D, E] -> [128(p), DT, E] with d = dt*128 + p, fp16
    wqT = np.ascontiguousarray(
        Wq.T.reshape(DT, 128, E).transpose(1, 0, 2)).astype(np.float16)
    # Wk: [128(ki), D2T, 2, E] fp8 with d = 2*(c*128 + ki) + i, scaled
    wk8 = np.ascontiguousarray(
        (Wk.T * WSC).astype(np.float32).reshape(D2T, 128, 2, E)
        .transpose(1, 0, 2, 3)).astype(e4m3)
    weT = np.ascontiguousarray(
        We.reshape(ET, 128).T).astype(np.float16)

    in_maps = []
    for c in range(NCORES):
        bs = slice(c * BL, (c + 1) * BL)
        in_maps.append({
            "keys": np.ascontiguousarray(keys[bs]),
            "queryT16": np.ascontiguousarray(
                query[bs].T.reshape(DT, 128, BL).transpose(1, 0, 2)
            ).astype(np.float16),
            "wk8": wk8,
            "wqT16h0": np.ascontiguousarray(wqT[:, :, 0:512]),
            "wqT16h1": np.ascontiguousarray(wqT[:, :, 512:1024]),
            "weT16": weT,
        })

    nc = _get_nc()
    res = run_bass_kernel_spmd(nc, in_maps, list(range(NCORES)), trace=_trace)
    out = np.concatenate([res.results[c]["out"] for c in range(NCORES)], axis=0)
    if _return_raw:
        return out, res
    return out
